# revision 1
# baseline (speedup 1.0000x reference)
"""Trainium2 Bass kernel: single-head attention with QKV projections.

Problem (hardcoded): q/k/v [4,2048,1024] fp32, W_q/W_k/W_v [1024,1024] fp32;
out = softmax((x@Wq^T)(x@Wk^T)^T/32) @ (x@Wv^T), fp32 [4,2048,1024].

Sharding: 8 cores = 4 batches x 2 query-halves, pair-collective K/V
exchange with a permutation-invariant key layout.

Key insight: softmax-attention is invariant to the ordering of keys, so each
core keeps ITS OWN K^T/V half in SBUF as k-tiles 0..7 and places the PEER
half (from a pair AllGather) as k-tiles 8..15 — regardless of which global
half it owns. The peer block inside the gathered buffer is selected with a
partition_id-derived dynamic offset.

K and V are exchanged in two separate collectives so the K exchange (needed
first, by S^T tiles 8..15) pipelines ahead of the V exchange (needed last,
by the AV accumulation).

Per-core PE work: 896 N=512 matmuls + 128 N=1 (v1: 1152 + 128).
"""

import numpy as np
import ml_dtypes

P = 128
D = 1024
E = 1024
QL = 1024
KL = 2048
KH = 1024
DT, ET, QT, KT = D // P, E // P, QL // P, KL // P
KHT = KH // P

_CACHE = {}


def _build_nc():
    from contextlib import ExitStack

    import concourse.bass as bass
    import concourse.mybir as mybir
    import concourse.tile as tile
    from concourse import bacc

    BF = mybir.dt.bfloat16
    F32 = mybir.dt.float32
    AFT = mybir.ActivationFunctionType

    nc = bacc.Bacc("TRN2", target_bir_lowering=False, debug=False,
                   enable_asserts=False, num_devices=8)

    qinT = nc.dram_tensor("qinT", [D, QL], BF, kind="ExternalInput").ap()
    kinT = nc.dram_tensor("kinT", [D, KH], BF, kind="ExternalInput").ap()
    vinT = nc.dram_tensor("vinT", [D, KH], BF, kind="ExternalInput").ap()
    wqT = nc.dram_tensor("wqT", [D, E], BF, kind="ExternalInput").ap()
    wkT = nc.dram_tensor("wkT", [D, E], BF, kind="ExternalInput").ap()
    wvT = nc.dram_tensor("wvT", [D, E], BF, kind="ExternalInput").ap()
    out = nc.dram_tensor("out", [QL, E], F32, kind="ExternalOutput").ap()

    RG = [[0, 1], [2, 3], [4, 5], [6, 7]]

    with tile.TileContext(nc) as tc, ExitStack() as ctx:
        wpool = ctx.enter_context(tc.tile_pool(name="w", bufs=2))
        apool = ctx.enter_context(tc.tile_pool(name="acts", bufs=2))
        qt_pool = ctx.enter_context(tc.tile_pool(name="qT", bufs=1))
        kt_pool = ctx.enter_context(tc.tile_pool(name="kT", bufs=1))
        v_pool = ctx.enter_context(tc.tile_pool(name="V", bufs=1))
        pt_pool = ctx.enter_context(tc.tile_pool(name="pT", bufs=1))
        o_pool = ctx.enter_context(tc.tile_pool(name="o", bufs=3))
        small = ctx.enter_context(tc.tile_pool(name="small", bufs=1))
        r_pool = ctx.enter_context(tc.tile_pool(name="r", bufs=2))
        ps = ctx.enter_context(tc.tile_pool(name="ps", bufs=3, space="PSUM"))
        ps_s = ctx.enter_context(tc.tile_pool(name="ps_s", bufs=2, space="PSUM"))
        dram = ctx.enter_context(tc.tile_pool(name="dram", bufs=1, space="DRAM"))

        ones_t = small.tile([P, 1], BF, tag="ones")
        nc.vector.memset(ones_t, 1.0)

        qT_sb = qt_pool.tile([P, ET, QL], BF, tag="qT")
        kT_sb = kt_pool.tile([P, ET, KL], BF, tag="kT")
        V_sb = v_pool.tile([P, KT, E], BF, tag="V")
        pT_sb = pt_pool.tile([P, KT, QL], BF, tag="pT")

        cc_in_k = dram.tile([KHT, P, KH], BF, tag="cc_in_k")
        cc_out_k = dram.tile([2 * KHT, P, KH], BF, tag="cc_out_k")
        cc_in_v = dram.tile([KHT, P, E], BF, tag="cc_in_v")
        cc_out_v = dram.tile([2 * KHT, P, E], BF, tag="cc_out_v")

        # ---- input DMAs, interleaved across the two HWDGE rings ----
        wk_t = [wpool.tile([P, E], BF, tag=f"w{dt}", name=f"wk{dt}")
                for dt in range(DT)]
        kin_t = [apool.tile([P, KH], BF, tag=f"a{dt}", name=f"kin{dt}")
                 for dt in range(DT)]
        for dt in range(DT):
            nc.sync.dma_start(out=wk_t[dt], in_=wkT[dt * P:(dt + 1) * P, :])
            nc.scalar.dma_start(out=kin_t[dt], in_=kinT[dt * P:(dt + 1) * P, :])
        wv_t = [wpool.tile([P, E], BF, tag=f"w{dt}", name=f"wv{dt}")
                for dt in range(DT)]
        vin_t = [apool.tile([P, KH], BF, tag=f"a{dt}", name=f"vin{dt}")
                 for dt in range(DT)]
        for dt in range(DT):
            nc.sync.dma_start(out=wv_t[dt], in_=wvT[dt * P:(dt + 1) * P, :])
            nc.scalar.dma_start(out=vin_t[dt], in_=vinT[dt * P:(dt + 1) * P, :])

        # ---- Phase B': local K^T half -> kT_sb k-tiles 0..7 ----
        for et in range(ET):
            acc = ps.tile([P, KH], F32, tag="ps")
            for dt in range(DT):
                w_sl = wk_t[dt][:, et * P:(et + 1) * P]
                for c in range(2):
                    nc.tensor.matmul(
                        acc[:, c * 512:(c + 1) * 512], w_sl,
                        kin_t[dt][:, c * 512:(c + 1) * 512],
                        start=(dt == 0), stop=(dt == DT - 1))
            nc.vector.tensor_copy(kT_sb[:, et, 0:KH], acc)
            nc.scalar.dma_start(out=cc_in_k[et], in_=kT_sb[:, et, 0:KH])
        nc.gpsimd.collective_compute(
            "AllGather", mybir.AluOpType.bypass, replica_groups=RG,
            ins=[cc_in_k.opt()], outs=[cc_out_k.opt()])

        # ---- Phase C': local V half -> V_sb k-tiles 0..7 ----
        for kt in range(KHT):
            acc = ps.tile([P, E], F32, tag="ps")
            for dt in range(DT):
                v_sl = vin_t[dt][:, kt * P:(kt + 1) * P]
                for c in range(2):
                    nc.tensor.matmul(
                        acc[:, c * 512:(c + 1) * 512], v_sl,
                        wv_t[dt][:, c * 512:(c + 1) * 512],
                        start=(dt == 0), stop=(dt == DT - 1))
            nc.vector.tensor_copy(V_sb[:, kt, :], acc)
            nc.scalar.dma_start(out=cc_in_v[kt], in_=V_sb[:, kt, :])
        nc.gpsimd.collective_compute(
            "AllGather", mybir.AluOpType.bypass, replica_groups=RG,
            ins=[cc_in_v.opt()], outs=[cc_out_v.opt()])

        # ---- Phase A: Q^T = WqT.T @ qinT (overlaps the collectives) ----
        wq_t = [wpool.tile([P, E], BF, tag=f"w{dt}", name=f"wq{dt}")
                for dt in range(DT)]
        qin_t = [apool.tile([P, QL], BF, tag=f"a{dt}", name=f"qin{dt}")
                 for dt in range(DT)]
        for dt in range(DT):
            nc.sync.dma_start(out=wq_t[dt], in_=wqT[dt * P:(dt + 1) * P, :])
            nc.scalar.dma_start(out=qin_t[dt], in_=qinT[dt * P:(dt + 1) * P, :])
        for et in range(ET):
            acc = ps.tile([P, QL], F32, tag="ps")
            for dt in range(DT):
                w_sl = wq_t[dt][:, et * P:(et + 1) * P]
                for c in range(2):
                    nc.tensor.matmul(
                        acc[:, c * 512:(c + 1) * 512], w_sl,
                        qin_t[dt][:, c * 512:(c + 1) * 512],
                        start=(dt == 0), stop=(dt == DT - 1))
            nc.vector.tensor_copy(qT_sb[:, et, :], acc)

        # ---- unpack the PEER halves into k-tiles 8..15 ----
        # peer block start: 8 if I'm the even rank of the pair, else 0
        pid = nc.sync.partition_id()
        peer_start = 8 - (pid % 2) * 8
        src_k = cc_out_k[bass.ds(peer_start, KHT)].rearrange("t p c -> p t c")
        nc.sync.dma_start(out=kT_sb[:, :, KH:KL], in_=src_k)
        src_v = cc_out_v[bass.ds(peer_start, KHT)].rearrange("t p c -> p t c")
        nc.sync.dma_start(out=V_sb[:, KHT:KT, :], in_=src_v)

        # ---- Phase D: S^T ; P^T = exp(S^T/32)  (local k-tiles first) ----
        for kt in range(KT):
            acc = ps.tile([P, QL], F32, tag="ps")
            for et in range(ET):
                k_sl = kT_sb[:, et, kt * P:(kt + 1) * P]
                for c in range(2):
                    nc.tensor.matmul(
                        acc[:, c * 512:(c + 1) * 512], k_sl,
                        qT_sb[:, et, c * 512:(c + 1) * 512],
                        start=(et == 0), stop=(et == ET - 1))
            nc.scalar.activation(pT_sb[:, kt, :], acc, AFT.Exp, scale=1.0 / 32.0)

        # ---- Phase E: O' = P^T.T @ V ; s = P^T.T @ 1 ; out = O'/s ----
        for qt in range(QT):
            acc = ps.tile([P, E], F32, tag="ps")
            ssum = ps_s.tile([P, 1], F32, tag="ps_s")
            for kt in range(KT):
                p_sl = pT_sb[:, kt, qt * P:(qt + 1) * P]
                # ssum first: the softmax denominator completes two matmuls
                # before the accumulation does, hiding the reciprocal
                nc.tensor.matmul(ssum[:, 0:1], p_sl, ones_t[:, 0:1],
                                 start=(kt == 0), stop=(kt == KT - 1))
                for c in range(2):
                    nc.tensor.matmul(
                        acc[:, c * 512:(c + 1) * 512], p_sl,
                        V_sb[:, kt, c * 512:(c + 1) * 512],
                        start=(kt == 0), stop=(kt == KT - 1))
            r_t = r_pool.tile([P, 1], F32, tag="r")
            nc.vector.reciprocal(r_t, ssum[:, 0:1])
            o_t = o_pool.tile([P, E], F32, tag="o")
            nc.scalar.activation(o_t[:, 0:512], acc[:, 0:512], AFT.Copy,
                                 scale=r_t[:, 0:1])
            nc.sync.dma_start(out=out[qt * P:(qt + 1) * P, 0:512],
                              in_=o_t[:, 0:512])
            nc.scalar.activation(o_t[:, 512:1024], acc[:, 512:1024], AFT.Copy,
                                 scale=r_t[:, 0:1])
            nc.scalar.dma_start(out=out[qt * P:(qt + 1) * P, 512:1024],
                                in_=o_t[:, 512:1024])

    nc.compile()
    return nc


def _get_nc():
    if "nc" not in _CACHE:
        _CACHE["nc"] = _build_nc()
    return _CACHE["nc"]


def make_in_maps(q, k, v, W_q, W_k, W_v):
    bf = ml_dtypes.bfloat16
    wqT = np.asarray(W_q, dtype=np.float32).T.astype(bf)
    wkT = np.asarray(W_k, dtype=np.float32).T.astype(bf)
    wvT = np.asarray(W_v, dtype=np.float32).T.astype(bf)
    in_maps = []
    for c in range(8):
        b, h = c // 2, c % 2
        sl = slice(h * 1024, (h + 1) * 1024)
        in_maps.append({
            "qinT": np.asarray(q[b, sl, :], dtype=np.float32).T.astype(bf),
            "kinT": np.asarray(k[b, sl, :], dtype=np.float32).T.astype(bf),
            "vinT": np.asarray(v[b, sl, :], dtype=np.float32).T.astype(bf),
            "wqT": wqT, "wkT": wkT, "wvT": wvT,
        })
    return in_maps


def kernel(**inputs):
    from concourse import bass_utils

    q = np.asarray(inputs["q_input"], dtype=np.float32)
    k = np.asarray(inputs["k_input"], dtype=np.float32)
    v = np.asarray(inputs["v_input"], dtype=np.float32)

    nc = _get_nc()
    in_maps = make_in_maps(q, k, v, inputs["W_q"], inputs["W_k"], inputs["W_v"])

    res = None
    for attempt in range(3):
        try:
            res = bass_utils.run_bass_kernel_spmd(nc, in_maps,
                                                  core_ids=list(range(8)))
            break
        except Exception:
            if attempt == 2:
                raise

    full = np.empty((4, 2048, 1024), dtype=np.float32)
    for c in range(8):
        b, h = c // 2, c % 2
        full[b, h * 1024:(h + 1) * 1024, :] = res.results[c]["out"]
    return full



# revision 3
# speedup vs baseline: 1.1243x; 1.1243x over previous
"""Trainium2 Bass kernel: single-head attention with QKV projections.

Problem (hardcoded): q/k/v [4,2048,1024] fp32, W_q/W_k/W_v [1024,1024] fp32;
out = softmax((x@Wq^T)(x@Wk^T)^T/32) @ (x@Wv^T), fp32 [4,2048,1024].

Sharding: 8 cores = 4 batches x 2 query-halves, pair-collective K/V
exchange with a permutation-invariant key layout.

v2: the S = Q^T K matmul (the largest single phase) runs in fp8-e4m3
DoubleRow mode (contraction 256/instr, ~1.75x ALU rate).  W_q/W_k are
scaled by 32 on the host so Q,K land in e4m3's normal range (std ~18.5,
max ~103 < 240); exp() folds the 1/(32*32*32) back in.  The V/attention
path stays bf16: CPU-exact simulation puts this config at rel err
1.77e-2 vs the 2e-2 gate, while any fp8 quantization of the V path or
projections busts the budget.

Other changes vs v1:
- psum->SBUF copies of Q^T/K^T/V split Vector/Scalar halves so the
  3-deep PSUM pool never starves the PE (was a 432ns stall per 3 tiles).
- ~200 tiny warm-up matmuls issued before the input DMAs land keep the
  PE HAM clock-gate at 8/8 so the real stream starts at 2.4 GHz.
- K^T collective exchanged in fp8 (half the bytes of v1).
"""

import numpy as np
import ml_dtypes

P = 128
D = 1024
E = 1024
QL = 1024
KL = 2048
KH = 1024
DT, ET, QT, KT = D // P, E // P, QL // P, KL // P
KHT = KH // P

N_WARMUP_MM = 200

_CACHE = {}


def _build_nc():
    from contextlib import ExitStack

    import concourse.bass as bass
    import concourse.mybir as mybir
    import concourse.tile as tile
    from concourse import bacc

    BF = mybir.dt.bfloat16
    F8 = mybir.dt.float8e4
    F32 = mybir.dt.float32
    AFT = mybir.ActivationFunctionType
    DR = mybir.MatmulPerfMode.DoubleRow

    nc = bacc.Bacc("TRN2", target_bir_lowering=False, debug=False,
                   enable_asserts=False, num_devices=8)

    qinT = nc.dram_tensor("qinT", [D, QL], BF, kind="ExternalInput").ap()
    kinT = nc.dram_tensor("kinT", [D, KH], BF, kind="ExternalInput").ap()
    vinT = nc.dram_tensor("vinT", [D, KH], BF, kind="ExternalInput").ap()
    wqT = nc.dram_tensor("wqT", [D, E], BF, kind="ExternalInput").ap()
    wkT = nc.dram_tensor("wkT", [D, E], BF, kind="ExternalInput").ap()
    wvT = nc.dram_tensor("wvT", [D, E], BF, kind="ExternalInput").ap()
    out = nc.dram_tensor("out", [QL, E], F32, kind="ExternalOutput").ap()

    RG = [[0, 1], [2, 3], [4, 5], [6, 7]]

    with tile.TileContext(nc) as tc, ExitStack() as ctx:
        wpool = ctx.enter_context(tc.tile_pool(name="w", bufs=2))
        apool = ctx.enter_context(tc.tile_pool(name="acts", bufs=2))
        qt_pool = ctx.enter_context(tc.tile_pool(name="qT", bufs=1))
        kt_pool = ctx.enter_context(tc.tile_pool(name="kT", bufs=1))
        v_pool = ctx.enter_context(tc.tile_pool(name="V", bufs=1))
        pt_pool = ctx.enter_context(tc.tile_pool(name="pT", bufs=1))
        o_pool = ctx.enter_context(tc.tile_pool(name="o", bufs=3))
        small = ctx.enter_context(tc.tile_pool(name="small", bufs=1))
        r_pool = ctx.enter_context(tc.tile_pool(name="r", bufs=2))
        ps = ctx.enter_context(tc.tile_pool(name="ps", bufs=3, space="PSUM"))
        ps_s = ctx.enter_context(tc.tile_pool(name="ps_s", bufs=2, space="PSUM"))
        dram = ctx.enter_context(tc.tile_pool(name="dram", bufs=1, space="DRAM"))

        ones_t = small.tile([P, 1], BF, tag="ones")
        nc.vector.memset(ones_t, 1.0)
        warm_in = small.tile([P, 64], BF, tag="warm")
        nc.vector.memset(warm_in, 0.125)

        # ---- PE warm-up: keep HAM at 8/8 while the input DMAs land ----
        warm_ps = ps.tile([P, QL], F32, tag="ps")
        for _ in range(N_WARMUP_MM):
            nc.tensor.matmul(warm_ps[0:1, 0:64], warm_in[:, 0:1],
                             warm_in[:, 0:64], start=True, stop=True)

        qT_sb = qt_pool.tile([P, ET, QL], F8, tag="qT")
        kT_sb = kt_pool.tile([P, ET, KL], F8, tag="kT")
        V_sb = v_pool.tile([P, KT, E], BF, tag="V")
        pT_sb = pt_pool.tile([P, KT, QL], BF, tag="pT")

        cc_in_k = dram.tile([KHT, P, KH], F8, tag="cc_in_k")
        cc_out_k = dram.tile([2 * KHT, P, KH], F8, tag="cc_out_k")
        cc_in_v = dram.tile([KHT, P, E], BF, tag="cc_in_v")
        cc_out_v = dram.tile([2 * KHT, P, E], BF, tag="cc_out_v")

        # ---- input DMAs, interleaved across the two HWDGE rings ----
        wk_t = [wpool.tile([P, E], BF, tag=f"w{dt}", name=f"wk{dt}")
                for dt in range(DT)]
        kin_t = [apool.tile([P, KH], BF, tag=f"a{dt}", name=f"kin{dt}")
                 for dt in range(DT)]
        for dt in range(DT):
            nc.sync.dma_start(out=wk_t[dt], in_=wkT[dt * P:(dt + 1) * P, :])
            nc.scalar.dma_start(out=kin_t[dt], in_=kinT[dt * P:(dt + 1) * P, :])
        wv_t = [wpool.tile([P, E], BF, tag=f"w{dt}", name=f"wv{dt}")
                for dt in range(DT)]
        vin_t = [apool.tile([P, KH], BF, tag=f"a{dt}", name=f"vin{dt}")
                 for dt in range(DT)]
        for dt in range(DT):
            nc.sync.dma_start(out=wv_t[dt], in_=wvT[dt * P:(dt + 1) * P, :])
            nc.scalar.dma_start(out=vin_t[dt], in_=vinT[dt * P:(dt + 1) * P, :])

        # ---- Phase B': local K^T half -> kT_sb k-tiles 0..7 (fp8) ----
        for et in range(ET):
            acc = ps.tile([P, KH], F32, tag="ps")
            for dt in range(DT):
                w_sl = wk_t[dt][:, et * P:(et + 1) * P]
                for c in range(2):
                    nc.tensor.matmul(
                        acc[:, c * 512:(c + 1) * 512], w_sl,
                        kin_t[dt][:, c * 512:(c + 1) * 512],
                        start=(dt == 0), stop=(dt == DT - 1))
            nc.vector.tensor_copy(kT_sb[:, et, 0:512], acc[:, 0:512])
            nc.scalar.activation(kT_sb[:, et, 512:KH], acc[:, 512:KH], AFT.Copy)
            nc.scalar.dma_start(out=cc_in_k[et], in_=kT_sb[:, et, 0:KH])
        nc.gpsimd.collective_compute(
            "AllGather", mybir.AluOpType.bypass, replica_groups=RG,
            ins=[cc_in_k.opt()], outs=[cc_out_k.opt()])

        # ---- Phase C': local V half -> V_sb k-tiles 0..7 ----
        for kt in range(KHT):
            acc = ps.tile([P, E], F32, tag="ps")
            for dt in range(DT):
                v_sl = vin_t[dt][:, kt * P:(kt + 1) * P]
                for c in range(2):
                    nc.tensor.matmul(
                        acc[:, c * 512:(c + 1) * 512], v_sl,
                        wv_t[dt][:, c * 512:(c + 1) * 512],
                        start=(dt == 0), stop=(dt == DT - 1))
            nc.vector.tensor_copy(V_sb[:, kt, 0:512], acc[:, 0:512])
            nc.scalar.activation(V_sb[:, kt, 512:E], acc[:, 512:E], AFT.Copy)
            nc.scalar.dma_start(out=cc_in_v[kt], in_=V_sb[:, kt, :])
        nc.gpsimd.collective_compute(
            "AllGather", mybir.AluOpType.bypass, replica_groups=RG,
            ins=[cc_in_v.opt()], outs=[cc_out_v.opt()])

        # ---- Phase A: Q^T = WqT.T @ qinT (overlaps the collectives) ----
        wq_t = [wpool.tile([P, E], BF, tag=f"w{dt}", name=f"wq{dt}")
                for dt in range(DT)]
        qin_t = [apool.tile([P, QL], BF, tag=f"a{dt}", name=f"qin{dt}")
                 for dt in range(DT)]
        for dt in range(DT):
            nc.sync.dma_start(out=wq_t[dt], in_=wqT[dt * P:(dt + 1) * P, :])
            nc.scalar.dma_start(out=qin_t[dt], in_=qinT[dt * P:(dt + 1) * P, :])
        for et in range(ET):
            acc = ps.tile([P, QL], F32, tag="ps")
            for dt in range(DT):
                w_sl = wq_t[dt][:, et * P:(et + 1) * P]
                for c in range(2):
                    nc.tensor.matmul(
                        acc[:, c * 512:(c + 1) * 512], w_sl,
                        qin_t[dt][:, c * 512:(c + 1) * 512],
                        start=(dt == 0), stop=(dt == DT - 1))
            nc.vector.tensor_copy(qT_sb[:, et, 0:512], acc[:, 0:512])
            nc.scalar.activation(qT_sb[:, et, 512:QL], acc[:, 512:QL], AFT.Copy)

        # ---- unpack the PEER halves into k-tiles 8..15 ----
        # peer block start: 8 if I'm the even rank of the pair, else 0
        pid = nc.sync.partition_id()
        peer_start = 8 - (pid % 2) * 8
        src_k = cc_out_k[bass.ds(peer_start, KHT)].rearrange("t p c -> p t c")
        nc.sync.dma_start(out=kT_sb[:, :, KH:KL], in_=src_k)
        src_v = cc_out_v[bass.ds(peer_start, KHT)].rearrange("t p c -> p t c")
        nc.sync.dma_start(out=V_sb[:, KHT:KT, :], in_=src_v)

        # ---- Phase D: S^T (fp8 DoubleRow); P^T = exp(S^T/32768) ----
        # Q' = 32Q, K' = 32K  ->  S'/32768 = QK/32
        for kt in range(KT):
            acc = ps.tile([P, QL], F32, tag="ps")
            for t in range(ET // 2):
                k_sl = kT_sb[:, 2 * t:2 * t + 2, kt * P:(kt + 1) * P]
                for c in range(2):
                    nc.tensor.matmul(
                        acc[:, c * 512:(c + 1) * 512], k_sl,
                        qT_sb[:, 2 * t:2 * t + 2, c * 512:(c + 1) * 512],
                        start=(t == 0), stop=(t == ET // 2 - 1),
                        perf_mode=DR)
            nc.scalar.activation(pT_sb[:, kt, :], acc, AFT.Exp,
                                 scale=1.0 / 32768.0)

        # ---- Phase E: O' = P^T.T @ V ; s = P^T.T @ 1 ; out = O'/s ----
        for qt in range(QT):
            acc = ps.tile([P, E], F32, tag="ps")
            ssum = ps_s.tile([P, 1], F32, tag="ps_s")
            for kt in range(KT):
                p_sl = pT_sb[:, kt, qt * P:(qt + 1) * P]
                # ssum first: the softmax denominator completes two matmuls
                # before the accumulation does, hiding the reciprocal
                nc.tensor.matmul(ssum[:, 0:1], p_sl, ones_t[:, 0:1],
                                 start=(kt == 0), stop=(kt == KT - 1))
                for c in range(2):
                    nc.tensor.matmul(
                        acc[:, c * 512:(c + 1) * 512], p_sl,
                        V_sb[:, kt, c * 512:(c + 1) * 512],
                        start=(kt == 0), stop=(kt == KT - 1))
            r_t = r_pool.tile([P, 1], F32, tag="r")
            nc.vector.reciprocal(r_t, ssum[:, 0:1])
            o_t = o_pool.tile([P, E], F32, tag="o")
            nc.scalar.activation(o_t[:, 0:512], acc[:, 0:512], AFT.Copy,
                                 scale=r_t[:, 0:1])
            nc.sync.dma_start(out=out[qt * P:(qt + 1) * P, 0:512],
                              in_=o_t[:, 0:512])
            nc.scalar.activation(o_t[:, 512:1024], acc[:, 512:1024], AFT.Copy,
                                 scale=r_t[:, 0:1])
            nc.scalar.dma_start(out=out[qt * P:(qt + 1) * P, 512:1024],
                                in_=o_t[:, 512:1024])

    nc.compile()
    return nc


def _get_nc():
    if "nc" not in _CACHE:
        _CACHE["nc"] = _build_nc()
    return _CACHE["nc"]


def make_in_maps(q, k, v, W_q, W_k, W_v):
    bf = ml_dtypes.bfloat16
    wqT = (np.asarray(W_q, dtype=np.float32) * 32.0).T.astype(bf)
    wkT = (np.asarray(W_k, dtype=np.float32) * 32.0).T.astype(bf)
    wvT = np.asarray(W_v, dtype=np.float32).T.astype(bf)
    in_maps = []
    for c in range(8):
        b, h = c // 2, c % 2
        sl = slice(h * 1024, (h + 1) * 1024)
        in_maps.append({
            "qinT": np.asarray(q[b, sl, :], dtype=np.float32).T.astype(bf),
            "kinT": np.asarray(k[b, sl, :], dtype=np.float32).T.astype(bf),
            "vinT": np.asarray(v[b, sl, :], dtype=np.float32).T.astype(bf),
            "wqT": wqT, "wkT": wkT, "wvT": wvT,
        })
    return in_maps


def kernel(**inputs):
    from concourse import bass_utils

    q = np.asarray(inputs["q_input"], dtype=np.float32)
    k = np.asarray(inputs["k_input"], dtype=np.float32)
    v = np.asarray(inputs["v_input"], dtype=np.float32)

    nc = _get_nc()
    in_maps = make_in_maps(q, k, v, inputs["W_q"], inputs["W_k"], inputs["W_v"])

    res = None
    for attempt in range(3):
        try:
            res = bass_utils.run_bass_kernel_spmd(nc, in_maps,
                                                  core_ids=list(range(8)))
            break
        except Exception:
            if attempt == 2:
                raise
    full = np.empty((4, 2048, 1024), dtype=np.float32)
    for c in range(8):
        b, h = c // 2, c % 2
        full[b, h * 1024:(h + 1) * 1024, :] = res.results[c]["out"]
    return full
